# revision 1
# baseline (speedup 1.0000x reference)
"""Multi-head attention (double-softmax) Trainium2 kernel, 8-core SPMD.

Problem: B=2, S=2048, D=1024, H=16 heads (dh=64), fp32, torch-Linear
projections, logits = qp @ kp.T, score = softmax(softmax(logits)/8),
out = (score @ vp) concat -> @ Wo.T + bo.

Sharding: core c in 0..7 handles batch b = c//4 and head-group g = c%4
(4 heads = 256 projection dims). Each core computes a partial output
[S, D] (its heads' contribution through Wo); host sums groups of 4 and
adds bo.

Per-core device algorithm (all matmul operands fp16; PSUM fp32):
  qpT/kpT [j,t] = WxT.T @ xT   (x fed transposed from host, fp16)
  vpT     [e,t] likewise; vp = DMA-xbar-transpose(vpT) -> [t,e]
  per head hh, per ti-tile:
    L [ti,tj] = qpT_h.T @ kpT_h          (PSUM, fp32)
    E1 = exp(L)            (ACT, bf16, fused row-sum s1)
    E2 = exp(E1 * 1/(8 s1)) (ACT, fp16, fused row-sum s2)
    F  = E2 * (1/s2)        (DVE, fp16)  == final attention weights
    FT = DMA-xbar-transpose(F)
  U [e, ti] ... actually att[ti] via U = sum_tj vp.T @ F.T per ti-chunk
  attT [j, ti] collected; partial out = attT.T @ woT  (+host bo)
"""

import sys

if "/opt/trn_rl_repo" not in sys.path:
    sys.path.insert(0, "/opt/trn_rl_repo")

import numpy as np

import concourse.bacc as bacc
import concourse.mybir as mybir
import concourse.tile as tile
from concourse import bass_utils

F32 = mybir.dt.float32
F16 = mybir.dt.float16
BF16 = mybir.dt.bfloat16
AF = mybir.ActivationFunctionType
OP = mybir.AluOpType

P = 128          # partitions
S = 2048         # sequence
D = 1024         # model dim
JC = 256         # projection dims per core (4 heads x 64)
NT = S // P      # 16 t-tiles
KD = D // P      # 8 d-tiles
TC = S // 512    # 4 512-chunks
JT = JC // P     # 2 j-tiles
NH = 4           # heads per core
DH = 64          # head dim

_NC_CACHE = {}


def build():
    if "nc" in _NC_CACHE:
        return _NC_CACHE["nc"]
    nc = bacc.Bacc("TRN2", target_bir_lowering=False, debug=False)

    qT = nc.dram_tensor("qT", [D, S], F16, kind="ExternalInput")
    kT = nc.dram_tensor("kT", [D, S], F16, kind="ExternalInput")
    vT = nc.dram_tensor("vT", [D, S], F16, kind="ExternalInput")
    wqT = nc.dram_tensor("wqT", [D, JC], F16, kind="ExternalInput")
    wkT = nc.dram_tensor("wkT", [D, JC], F16, kind="ExternalInput")
    wvT = nc.dram_tensor("wvT", [D, JC], F16, kind="ExternalInput")
    woT = nc.dram_tensor("woT", [JC, D], F16, kind="ExternalInput")
    bq = nc.dram_tensor("bq", [P, JT], F32, kind="ExternalInput")
    bk = nc.dram_tensor("bk", [P, JT], F32, kind="ExternalInput")
    bv = nc.dram_tensor("bv", [P, JT], F32, kind="ExternalInput")
    out = nc.dram_tensor("out", [S, D], F32, kind="ExternalOutput")

    with tile.TileContext(nc) as tc:
        with (
            tc.tile_pool(name="wpool", bufs=1) as wpool,
            tc.tile_pool(name="xstream", bufs=2) as xstream,
            tc.tile_pool(name="proj", bufs=1) as proj,
            tc.tile_pool(name="work", bufs=3) as work,
            tc.tile_pool(name="work2", bufs=2) as work2,
            tc.tile_pool(name="ftp", bufs=3) as ftp,
            tc.tile_pool(name="stats", bufs=1) as stats,
            tc.tile_pool(name="outp", bufs=2) as outp,
            tc.tile_pool(name="ps_l", bufs=3, space="PSUM") as ps_l,
            tc.tile_pool(name="ps_v", bufs=1, space="PSUM") as ps_v,
            tc.tile_pool(name="ps_u", bufs=1, space="PSUM") as ps_u,
        ):  # noqa: indentation kept
            # ---- load weights & biases (SWDGE: keep SP ring for transposes) --
            w_sb = {}
            for name, t in (("q", wqT), ("k", wkT), ("v", wvT)):
                w = wpool.tile([P, KD, JC], F16, name=f"w_{name}")
                nc.gpsimd.dma_start(w[:], t[:].rearrange("(k p) j -> p k j", p=P))
                w_sb[name] = w
            wo_sb = wpool.tile([P, JT, D], F16, name="wo")
            nc.gpsimd.dma_start(wo_sb[:], woT[:].rearrange("(k p) j -> p k j", p=P))
            b_sb = {}
            for name, t in (("q", bq), ("k", bk), ("v", bv)):
                b = wpool.tile([P, JT], F32, name=f"b_{name}")
                nc.gpsimd.dma_start(b[:], t[:])
                b_sb[name] = b

            # ---- projections: pT[j, t] = w.T @ xT  (+bias) ----
            p_sb = {}  # [P, JT, S] fp16 (j/e on partitions)
            for name in ("q", "k", "v"):
                p_sb[name] = proj.tile([P, JT, S], F16, name=f"p_{name}")

            x_sb = {}

            def load_x(name, src_dram):
                x = xstream.tile([P, KD, S], F16, name="xT", tag="xT")
                r = src_dram[:].rearrange("(k p) t -> p k t", p=P)
                for kt in range(KD):
                    nc.gpsimd.dma_start(x[:, kt], r[:, kt])
                x_sb[name] = x

            def project_jt(name, jt, t4s=tuple(range(TC))):
                x = x_sb[name]
                for t4 in t4s:
                    psl = ps_l.tile([P, 1024], F32, name=f"pp_{name}_{jt}_{t4}",
                                    tag="L")
                    ps = psl[:, 0:512]
                    for kt in range(KD):
                        nc.tensor.matmul(
                            ps[:],
                            w_sb[name][:, kt, jt * P:(jt + 1) * P],
                            x[:, kt, t4 * 512:(t4 + 1) * 512],
                            start=(kt == 0), stop=(kt == KD - 1),
                        )
                    if name == "v":
                        # fold the (constant) second-softmax denominator:
                        # s2 = sum exp(score1/8) = 2048.129 +- 0.004 since
                        # score1 sums to 1 and is in [0,1].
                        nc.vector.tensor_scalar(
                            p_sb[name][:, jt, t4 * 512:(t4 + 1) * 512],
                            ps[:], b_sb[name][:, jt:jt + 1], 1.0 / 2048.129,
                            OP.add, OP.mult,
                        )
                    else:
                        nc.vector.tensor_scalar(
                            p_sb[name][:, jt, t4 * 512:(t4 + 1) * 512],
                            ps[:], b_sb[name][:, jt:jt + 1], None, OP.add,
                        )

            vp_sb = proj.tile([P, NT, JC], F16, name="vp")

            def emit_vp_transpose(jt):
                # vp = transpose(vpT): [P(t), NT, JC(e)] fp16
                nc.sync.dma_start_transpose(
                    vp_sb[:].rearrange("p n (j e) -> p n j e", j=JT)[:, :, jt, :],
                    p_sb["v"][:, jt, :],
                )

            # ---- attention state ----
            attT = proj.tile([P, JT, S], F16, name="attT")
            s1_sb = stats.tile([P, NT * NH], F32, name="s1")
            s2_sb = stats.tile([P, NT * NH], F32, name="s2")
            r1_sb = stats.tile([P, NT * NH], F32, name="r1")
            sc2_sb = stats.tile([P, NT * NH], F32, name="sc2")
            r2_sb = stats.tile([P, NT * NH], F32, name="r2")
            s1a_sb = stats.tile([P, NT * NH], F32, name="s1a")
            s1b_sb = stats.tile([P, NT * NH], F32, name="s1b")

            def emit_mt(t4, hp, hx, m4, ft):
                hh = 2 * hp + hx
                off = DH * hx
                mt = t4 * 4 + m4
                si = hh * NT + mt
                use_poly = (m4 % 2 == 1) and not (t4 == TC - 1 and hp == 1)
                e1 = work.tile([P, S], BF16, name="e1", tag="e1")
                for half in range(2):
                    lps = ps_l.tile([P, 1024], F32, name="L", tag="L")
                    for nc2 in range(2):
                        nch = half * 2 + nc2
                        nc.tensor.matmul(
                            lps[:, nc2 * 512:(nc2 + 1) * 512],
                            p_sb["q"][off:off + DH, hp, mt * P:(mt + 1) * P],
                            p_sb["k"][off:off + DH, hp,
                                      nch * 512:(nch + 1) * 512],
                            start=True, stop=True,
                        )
                    acc = (s1a_sb if half == 0 else s1b_sb)[:, si:si + 1]
                    nc.scalar.activation(
                        e1[:, half * 1024:(half + 1) * 1024], lps[:], AF.Exp,
                        accum_out=acc)
                nc.vector.scalar_tensor_tensor(
                    s1_sb[:, si:si + 1], s1a_sb[:, si:si + 1], 1.0,
                    s1b_sb[:, si:si + 1], OP.mult, OP.add)
                nc.vector.reciprocal(r1_sb[:, si:si + 1], s1_sb[:, si:si + 1])
                nc.vector.tensor_scalar(
                    sc2_sb[:, si:si + 1], r1_sb[:, si:si + 1],
                    0.125, None, OP.mult)
                if not use_poly:
                    # E2 transposed directly; the constant 1/s2 is folded
                    # into vp. Deferred one mt so the next mt's tiny recip
                    # chain stays ahead in engine FIFOs.
                    def emit_f(e1=e1, si=si, ft=ft, m4=m4):
                        e2 = work2.tile([P, S], F16, name="e2", tag="e2")
                        nc.scalar.activation(e2[:], e1[:], AF.Exp,
                                             scale=sc2_sb[:, si:si + 1])
                        nc.sync.dma_start_transpose(ft[:, m4], e2[:])
                    fq.append(emit_f)
                else:
                    # exp2 via deg-2 Taylor on DVE: exp(x) ~= 1 + x(1 + x/2)
                    # for x = E1*sc2 in [0, 1/8]. Offloads the ACT engine.
                    def emit_poly(e1=e1, si=si, ft=ft, m4=m4):
                        x = work2.tile([P, S], F16, name="px", tag="e2")
                        nc.vector.tensor_scalar(
                            x[:], e1[:], sc2_sb[:, si:si + 1], None, OP.mult)
                        w = work.tile([P, S], F16, name="pw", tag="f")
                        nc.vector.tensor_scalar(
                            w[:], x[:], 0.5, 1.0, OP.mult, OP.add)
                        u = work.tile([P, S], F16, name="pu", tag="e1")
                        nc.vector.tensor_mul(u[:], x[:], w[:])
                        e2p = work.tile([P, S], F16, name="pe2", tag="f")
                        nc.vector.tensor_scalar(
                            e2p[:], u[:], 1.0, None, OP.add)
                        nc.sync.dma_start_transpose(ft[:, m4], e2p[:])
                    fq.append(emit_poly)

            def make_u_emitters(t4, hp, fts):
                state = {}

                def emit_u_half(lo, hi, last):
                    vp = vp_sb
                    if "ups" not in state:
                        state["ups"] = ps_u.tile([P, 512], F32, name="U",
                                                 tag="U")
                    ups = state["ups"]
                    for kt in range(lo, hi):
                        for hx in range(2):
                            nc.tensor.matmul(
                                ups[hx * DH:(hx + 1) * DH, :],
                                vp[:, kt,
                                   hp * P + hx * DH:hp * P + (hx + 1) * DH],
                                fts[hx][:, :, kt, :],
                                start=(kt == 0), stop=(kt == NT - 1),
                                tile_position=(0, hx * DH),
                            )
                    if last:
                        nc.vector.tensor_copy(
                            attT[:, hp, t4 * 512:(t4 + 1) * 512], ups[:])

                return [lambda: emit_u_half(0, 8, False),
                        lambda: emit_u_half(8, NT, True)]

            def emit_v(t4, m4s=(0, 1, 2, 3)):
                for m4 in m4s:
                    mt = t4 * 4 + m4
                    for oc in range(2):
                        vps = ps_v.tile([P, 512], F32, name=f"V_{mt}_{oc}",
                                        tag="ps_v")
                        for jt in range(JT):
                            nc.tensor.matmul(
                                vps[:],
                                attT[:, jt, mt * P:(mt + 1) * P],
                                wo_sb[:, jt, oc * 512:(oc + 1) * 512],
                                start=(jt == 0), stop=(jt == JT - 1),
                            )
                        o = outp.tile([P, 512], F32, name="o", tag="o")
                        nc.vector.tensor_copy(o[:], vps[:])
                        nc.gpsimd.dma_start(
                            out[mt * P:(mt + 1) * P,
                                oc * 512:(oc + 1) * 512], o[:])

            def emit_group(t4, hp, pending):
                """Emit one (t4, head-pair) group's 8 mt pipelines.
                pending: deferred closures (U halves of prev group, V of
                prev tc) interleaved after early mts so the next group's
                L matmuls keep priority while PE slack still gets filled."""
                fts = []
                pi = 0
                for hx in range(2):
                    ft = ftp.tile([P, 4, NT, P], F16, name="ft", tag="ft")
                    fts.append(ft)
                    for m4 in range(4):
                        emit_mt(t4, hp, hx, m4, ft)
                        while len(fq) > 1:
                            fq.pop(0)()
                        if pi < len(pending):
                            pending[pi]()
                            pi += 1
                while pi < len(pending):
                    pending[pi]()
                    pi += 1
                return make_u_emitters(t4, hp, fts)

            fq = []  # deferred F emitters

            # ---- emission schedule (just-in-time projections) ----
            load_x("k", kT)
            load_x("q", qT)
            project_jt("k", 0)
            project_jt("q", 0, t4s=(0,))

            pend = [
                lambda: project_jt("k", 1, (0, 1)),
                lambda: project_jt("k", 1, (2, 3)),
                lambda: project_jt("q", 1, (0,)),
                lambda: load_x("v", vT),
            ]
            u_prev = emit_group(0, 0, pend)

            pend = [
                lambda: project_jt("q", 0, (1,)),
                lambda: project_jt("q", 1, (1,)),
                lambda: project_jt("v", 0, (0, 1)),
                lambda: project_jt("v", 0, (2, 3)),
                lambda: emit_vp_transpose(0),
                lambda: project_jt("v", 1, (0, 1)),
                lambda: project_jt("v", 1, (2, 3)),
                lambda: emit_vp_transpose(1),
                u_prev[0], u_prev[1],
            ]
            u_prev = emit_group(0, 1, pend)

            for t4, hp in [(t4, hp) for t4 in range(1, TC) for hp in range(2)]:
                pend = [u_prev[0], u_prev[1]]
                if hp == 0:
                    if t4 < TC - 1:
                        pend += [
                            lambda t=t4 + 1: project_jt("q", 0, (t,)),
                            lambda t=t4 + 1: project_jt("q", 1, (t,)),
                        ]
                else:
                    pend += [
                        lambda t=t4 - 1: emit_v(t, (0,)),
                        lambda t=t4 - 1: emit_v(t, (1,)),
                        lambda t=t4 - 1: emit_v(t, (2,)),
                        lambda t=t4 - 1: emit_v(t, (3,)),
                    ]
                u_prev = emit_group(t4, hp, pend)
            while fq:
                fq.pop(0)()
            for pu in u_prev:
                pu()
            emit_v(TC - 1)

    nc.compile()
    _NC_CACHE["nc"] = nc
    return nc


def _prep_core_inputs(q, k, v, Wq, bq, Wk, bk, Wv, bv, Wo, bo):
    """Host-side sharding: returns list of 8 input dicts."""
    in_maps = []
    xT = {}
    for b in range(2):
        xT[b] = {
            "qT": np.ascontiguousarray(q[b].T).astype(np.float16),
            "kT": np.ascontiguousarray(k[b].T).astype(np.float16),
            "vT": np.ascontiguousarray(v[b].T).astype(np.float16),
        }
    for c in range(8):
        b, g = c // 4, c % 4
        jsl = slice(JC * g, JC * (g + 1))
        m = dict(xT[b])
        m["wqT"] = np.ascontiguousarray(Wq[jsl].T).astype(np.float16)
        m["wkT"] = np.ascontiguousarray(Wk[jsl].T).astype(np.float16)
        m["wvT"] = np.ascontiguousarray(Wv[jsl].T).astype(np.float16)
        m["woT"] = np.ascontiguousarray(Wo[:, jsl].T).astype(np.float16)
        m["bq"] = np.ascontiguousarray(bq[jsl].reshape(JT, P).T).astype(np.float32)
        m["bk"] = np.ascontiguousarray(bk[jsl].reshape(JT, P).T).astype(np.float32)
        m["bv"] = np.ascontiguousarray(bv[jsl].reshape(JT, P).T).astype(np.float32)
        in_maps.append(m)
    return in_maps


def kernel(q, k, v, Wq, bq, Wk, bk, Wv, bv, Wo, bo, _trace=False, _result=[None]):
    q, k, v = (np.asarray(x, dtype=np.float32) for x in (q, k, v))
    Wq, bq, Wk, bk, Wv, bv, Wo, bo = (
        np.asarray(x, dtype=np.float32) for x in (Wq, bq, Wk, bk, Wv, bv, Wo, bo))
    nc = build()
    in_maps = _prep_core_inputs(q, k, v, Wq, bq, Wk, bk, Wv, bv, Wo, bo)
    res = bass_utils.run_bass_kernel_spmd(
        nc, in_maps, core_ids=list(range(8)), trace=_trace)
    _result[0] = res
    out = np.zeros((2, S, D), dtype=np.float32)
    for c in range(8):
        out[c // 4] += res.results[c]["out"]
    out += bo[None, None, :]
    return out



# revision 2
# speedup vs baseline: 1.0947x; 1.0947x over previous
"""Multi-head attention (double-softmax) Trainium2 kernel, 8-core SPMD.

Problem: B=2, S=2048, D=1024, H=16 heads (dh=64), fp32, torch-Linear
projections, logits = qp @ kp.T, score = softmax(softmax(logits)/8),
out = (score @ vp) concat -> @ Wo.T + bo.

Key algebraic simplification: the second softmax's input score1/8 lies in
[0, 1/8], so exp(x) ~= 1 + x with error <= x^2/2 <= 1/128 (relative to a
sum of 2048 terms ~ 1e-5 of the output).  With s2 = sum_j exp(score1/8)
= 2048.129 +- 0.004 (constant to ~2e-6 relative):

  out ~= [ vsum + (1/8) * score1 @ vp ] / s2 @ Wo.T + bo

where vsum = sum_t vp[t] is a rank-1 term identical for every query row.
The host computes the vsum term exactly (tiny GEMV); the device computes
only the score1 @ vp correction:

Per-core device algorithm (core c: batch b=c//4, head-group g=c%4, 4
heads x 64 = 256 projection dims; all matmuls fp16/bf16, PSUM fp32):
  qpT/kpT [j,t] = WxT.T @ xT + b  (fp16)
  vpT     [e,t] likewise (bf16); vp = DMA-xbar-transpose(vpT) -> [t,e]
  per (mt, h) tile (mt: 16 row-tiles of 128, h: 4 heads):
    L  [ti,tj] = qpT_h.T @ kpT_h          (PSUM fp32, 2x 1024 halves)
    E1 = exp(L)                            (ACT, bf16, fused row-sum s1)
    E1T = DMA-xbar-transpose(E1)           (bf16, [tj, ti])
    U  [ti,e]  = sum_kt E1T_kt.T @ vp_kt   (PSUM, ti on partitions!)
    att[ti,e]  = U * (1/s1[ti])            (DVE, per-partition scalar)
    attT = DMA-xbar-transpose(att)
    out[ti,:] += attT.T @ woT * 1/(8*s2)   (per 512-col chunk)
Host: out[b] = sum_cores + (v[b].sum(0) @ Wv.T + S*bv)/s2 @ Wo.T + bo.
"""

import sys

if "/opt/trn_rl_repo" not in sys.path:
    sys.path.insert(0, "/opt/trn_rl_repo")

import numpy as np

import concourse.bacc as bacc
import concourse.mybir as mybir
import concourse.tile as tile
from concourse import bass_utils

F32 = mybir.dt.float32
F16 = mybir.dt.float16
BF16 = mybir.dt.bfloat16
AF = mybir.ActivationFunctionType
OP = mybir.AluOpType

P = 128          # partitions
S = 2048         # sequence
D = 1024         # model dim
JC = 256         # projection dims per core (4 heads x 64)
NT = S // P      # 16 row-tiles
KD = D // P      # 8 d-subtiles
TC = S // 512    # 4 512-chunks
JT = JC // P     # 2 j-subtiles
NH = 4           # heads per core
DH = 64          # head dim
S2C = 2048.129   # constant second-softmax denominator
OUTC = 1.0 / (8.0 * S2C)

_NC_CACHE = {}


def build():
    if "nc" in _NC_CACHE:
        return _NC_CACHE["nc"]
    nc = bacc.Bacc("TRN2", target_bir_lowering=False, debug=False)

    qT = nc.dram_tensor("qT", [D, S], F16, kind="ExternalInput")
    kT = nc.dram_tensor("kT", [D, S], F16, kind="ExternalInput")
    vT = nc.dram_tensor("vT", [D, S], F16, kind="ExternalInput")
    wqT = nc.dram_tensor("wqT", [D, JC], F16, kind="ExternalInput")
    wkT = nc.dram_tensor("wkT", [D, JC], F16, kind="ExternalInput")
    wvT = nc.dram_tensor("wvT", [D, JC], F16, kind="ExternalInput")
    woT = nc.dram_tensor("woT", [JC, D], F16, kind="ExternalInput")
    bq = nc.dram_tensor("bq", [P, JT], F32, kind="ExternalInput")
    bk = nc.dram_tensor("bk", [P, JT], F32, kind="ExternalInput")
    bv = nc.dram_tensor("bv", [P, JT], F32, kind="ExternalInput")
    out = nc.dram_tensor("out", [S, D], F32, kind="ExternalOutput")

    with tile.TileContext(nc) as tc:
        with (
            tc.tile_pool(name="wpool", bufs=1) as wpool,
            tc.tile_pool(name="xstream", bufs=2) as xstream,
            tc.tile_pool(name="proj", bufs=1) as proj,
            tc.tile_pool(name="e1p", bufs=3) as e1p,
            tc.tile_pool(name="ftp", bufs=3) as ftp,
            tc.tile_pool(name="attp", bufs=2) as attp,
            tc.tile_pool(name="attTp", bufs=2) as attTp,
            tc.tile_pool(name="outp", bufs=2) as outp,
            tc.tile_pool(name="stats", bufs=1) as stats,
            tc.tile_pool(name="ps_l", bufs=3, space="PSUM") as ps_l,
            tc.tile_pool(name="ps_u", bufs=1, space="PSUM") as ps_u,
            tc.tile_pool(name="ps_v", bufs=1, space="PSUM") as ps_v,
        ):
            # ---- weights & biases (SWDGE: keep SP ring for transposes) ----
            w_sb = {}
            for name, t in (("q", wqT), ("k", wkT), ("v", wvT)):
                w = wpool.tile([P, KD, JC], F16, name=f"w_{name}")
                nc.gpsimd.dma_start(w[:], t[:].rearrange("(k p) j -> p k j", p=P))
                w_sb[name] = w
            wo_sb = wpool.tile([P, JT, D], F16, name="wo")
            nc.gpsimd.dma_start(wo_sb[:], woT[:].rearrange("(k p) j -> p k j", p=P))
            b_sb = {}
            for name, t in (("q", bq), ("k", bk), ("v", bv)):
                b = wpool.tile([P, JT], F32, name=f"b_{name}")
                nc.gpsimd.dma_start(b[:], t[:])
                b_sb[name] = b

            # ---- projections: pT[j, t] = w.T @ xT + b ----
            p_sb = {
                "q": proj.tile([P, JT, S], F16, name="p_q"),
                "k": proj.tile([P, JT, S], F16, name="p_k"),
                "v": proj.tile([P, JT, S], BF16, name="p_v"),
            }
            vp_sb = proj.tile([P, NT, JC], BF16, name="vp")

            x_sb = {}

            def load_x(name, src_dram):
                x = xstream.tile([P, KD, S], F16, name="xT", tag="xT")
                r = src_dram[:].rearrange("(k p) t -> p k t", p=P)
                for kt in range(KD):
                    nc.gpsimd.dma_start(x[:, kt], r[:, kt])
                x_sb[name] = x

            def project_jt(name, jt, t4s=tuple(range(TC))):
                x = x_sb[name]
                for t4 in t4s:
                    psl = ps_l.tile([P, 1024], F32, name=f"pp_{name}_{jt}_{t4}",
                                    tag="L")
                    ps = psl[:, 0:512]
                    for kt in range(KD):
                        nc.tensor.matmul(
                            ps[:],
                            w_sb[name][:, kt, jt * P:(jt + 1) * P],
                            x[:, kt, t4 * 512:(t4 + 1) * 512],
                            start=(kt == 0), stop=(kt == KD - 1),
                        )
                    nc.vector.tensor_scalar(
                        p_sb[name][:, jt, t4 * 512:(t4 + 1) * 512],
                        ps[:], b_sb[name][:, jt:jt + 1], None, OP.add,
                    )

            def emit_vp_transpose(jt):
                # vp = transpose(vpT): [P(t), NT, JC(e)] bf16
                nc.sync.dma_start_transpose(
                    vp_sb[:, :, jt * P:(jt + 1) * P],
                    p_sb["v"][:, jt, :],
                )

            # ---- attention state ----
            s1a_sb = stats.tile([P, NT * NH], F32, name="s1a")
            s1b_sb = stats.tile([P, NT * NH], F32, name="s1b")
            s1_sb = stats.tile([P, NT * NH], F32, name="s1")
            r1_sb = stats.tile([P, NT * NH], F32, name="r1")

            ft_tiles = {}
            ups_tiles = {}
            attT_tiles = {}

            def emit_l_e1(mt, h):
                """L = qp_h.T @ kp_h for row-tile mt; E1 = exp(L) + row sums;
                transpose E1 into ft[mt][:, h]."""
                si = mt * NH + h
                po = (h % 2) * DH
                jt = h // 2
                if h == 0:
                    ft_tiles[mt] = ftp.tile([P, NH, NT, P], BF16, name="ft",
                                            tag="ft")
                e1 = e1p.tile([P, S], BF16, name="e1", tag="e1")
                for half in range(2):
                    psl = ps_l.tile([P, 1024], F32, name="L", tag="L")
                    for nc2 in range(2):
                        nch = half * 2 + nc2
                        nc.tensor.matmul(
                            psl[:, nc2 * 512:(nc2 + 1) * 512],
                            p_sb["q"][po:po + DH, jt, mt * P:(mt + 1) * P],
                            p_sb["k"][po:po + DH, jt,
                                      nch * 512:(nch + 1) * 512],
                            start=True, stop=True,
                        )
                    acc = (s1a_sb if half == 0 else s1b_sb)[:, si:si + 1]
                    nc.scalar.activation(
                        e1[:, half * 1024:(half + 1) * 1024], psl[:], AF.Exp,
                        accum_out=acc)
                nc.vector.scalar_tensor_tensor(
                    s1_sb[:, si:si + 1], s1a_sb[:, si:si + 1], 1.0,
                    s1b_sb[:, si:si + 1], OP.mult, OP.add)
                nc.sync.dma_start_transpose(ft_tiles[mt][:, h], e1[:])

            def emit_u(mt, h):
                """U[ti, e_h] += sum_kt E1T_kt.T @ vp_kt (ti on partitions)."""
                if h == 0:
                    ups_tiles[mt] = ps_u.tile([P, JC], F32, name="U", tag="U")
                ups = ups_tiles[mt]
                ft = ft_tiles[mt]
                for kt in range(NT):
                    nc.tensor.matmul(
                        ups[:, h * DH:(h + 1) * DH],
                        ft[:, h, kt, :],
                        vp_sb[:, kt, h * DH:(h + 1) * DH],
                        start=(kt == 0), stop=(kt == NT - 1),
                    )

            def emit_norm(mt):
                """att = U * 1/s1 per row (partition scalar), then
                transpose to attT[e, ti] for the output projection."""
                ups = ups_tiles.pop(mt)
                att = attp.tile([P, JC], F16, name="att", tag="att")
                r_b = (r1_sb[:, mt * NH:(mt + 1) * NH]
                       .unsqueeze(2).to_broadcast([P, NH, DH]))
                nc.vector.tensor_mul(
                    att[:].rearrange("p (h e) -> p h e", h=NH),
                    ups[:].rearrange("p (h e) -> p h e", h=NH),
                    r_b)
                attT = attTp.tile([P, JT, P], F16, name="attT", tag="attT")
                nc.sync.dma_start_transpose(attT[:], att[:])
                attT_tiles[mt] = attT
                del ft_tiles[mt]

            def emit_outproj(mt):
                attT = attT_tiles.pop(mt)
                for oc in range(2):
                    vps = ps_v.tile([P, 512], F32, name=f"V_{mt}_{oc}",
                                    tag="ps_v")
                    for jt in range(JT):
                        nc.tensor.matmul(
                            vps[:],
                            attT[:, jt],
                            wo_sb[:, jt, oc * 512:(oc + 1) * 512],
                            start=(jt == 0), stop=(jt == JT - 1),
                        )
                    o = outp.tile([P, 512], F32, name="o", tag="o")
                    nc.vector.tensor_scalar(o[:], vps[:], OUTC, None, OP.mult)
                    nc.gpsimd.dma_start(
                        out[mt * P:(mt + 1) * P,
                            oc * 512:(oc + 1) * 512], o[:])

            # ---- emission schedule ----
            load_x("k", kT)
            load_x("q", qT)
            project_jt("k", 0)
            project_jt("k", 1)
            project_jt("q", 0, (0,))
            project_jt("q", 1, (0,))

            pend = [
                lambda: load_x("v", vT),
                lambda: project_jt("q", 0, (1,)),
                lambda: project_jt("q", 1, (1,)),
                lambda: project_jt("q", 0, (2,)),
                lambda: project_jt("v", 0, (0, 1)),
                lambda: project_jt("v", 0, (2, 3)),
                lambda: project_jt("v", 1, (0, 1)),
                lambda: project_jt("v", 1, (2, 3)),
                lambda: emit_vp_transpose(0),
                lambda: emit_vp_transpose(1),
                lambda: project_jt("q", 1, (2,)),
                lambda: project_jt("q", 0, (3,)),
                lambda: project_jt("q", 1, (3,)),
            ]

            for mt in range(NT):
                for h in range(NH):
                    if pend:
                        pend.pop(0)()
                    if mt >= 2:
                        emit_u(mt - 2, h)
                    emit_l_e1(mt, h)
                nc.vector.reciprocal(r1_sb[:, mt * NH:(mt + 1) * NH],
                                     s1_sb[:, mt * NH:(mt + 1) * NH])
                if mt >= 2:
                    emit_norm(mt - 2)
                if mt >= 3:
                    emit_outproj(mt - 3)
            for mt in (NT - 2, NT - 1):
                for h in range(NH):
                    emit_u(mt, h)
                emit_norm(mt)
            for mt in (NT - 3, NT - 2, NT - 1):
                emit_outproj(mt)

    nc.compile()
    _NC_CACHE["nc"] = nc
    return nc


def _prep_core_inputs(q, k, v, Wq, bq, Wk, bk, Wv, bv, Wo, bo):
    """Host-side sharding: returns list of 8 input dicts."""
    in_maps = []
    xT = {}
    for b in range(2):
        xT[b] = {
            "qT": np.ascontiguousarray(q[b].T).astype(np.float16),
            "kT": np.ascontiguousarray(k[b].T).astype(np.float16),
            "vT": np.ascontiguousarray(v[b].T).astype(np.float16),
        }
    for c in range(8):
        b, g = c // 4, c % 4
        jsl = slice(JC * g, JC * (g + 1))
        m = dict(xT[b])
        m["wqT"] = np.ascontiguousarray(Wq[jsl].T).astype(np.float16)
        m["wkT"] = np.ascontiguousarray(Wk[jsl].T).astype(np.float16)
        m["wvT"] = np.ascontiguousarray(Wv[jsl].T).astype(np.float16)
        m["woT"] = np.ascontiguousarray(Wo[:, jsl].T).astype(np.float16)
        m["bq"] = np.ascontiguousarray(bq[jsl].reshape(JT, P).T).astype(np.float32)
        m["bk"] = np.ascontiguousarray(bk[jsl].reshape(JT, P).T).astype(np.float32)
        m["bv"] = np.ascontiguousarray(bv[jsl].reshape(JT, P).T).astype(np.float32)
        in_maps.append(m)
    return in_maps


def kernel(q, k, v, Wq, bq, Wk, bk, Wv, bv, Wo, bo, _trace=False, _result=[None]):
    q, k, v = (np.asarray(x, dtype=np.float32) for x in (q, k, v))
    Wq, bq, Wk, bk, Wv, bv, Wo, bo = (
        np.asarray(x, dtype=np.float32) for x in (Wq, bq, Wk, bk, Wv, bv, Wo, bo))
    nc = build()
    in_maps = _prep_core_inputs(q, k, v, Wq, bq, Wk, bk, Wv, bv, Wo, bo)
    res = bass_utils.run_bass_kernel_spmd(
        nc, in_maps, core_ids=list(range(8)), trace=_trace)
    _result[0] = res
    out = np.zeros((2, S, D), dtype=np.float32)
    for c in range(8):
        out[c // 4] += res.results[c]["out"]
    # host-exact rank-1 term of the linearized second softmax (+ bias)
    for b in range(2):
        vsum = v[b].sum(0) @ Wv.T + S * bv
        out[b] += ((vsum / S2C) @ Wo.T + bo)[None, :]
    return out


# revision 16
# speedup vs baseline: 1.1527x; 1.0531x over previous
"""Multi-head attention (double-softmax) Trainium2 kernel, 8-core SPMD.

Problem: B=2, S=2048, D=1024, H=16 heads (dh=64), fp32, torch-Linear
projections, logits = qp @ kp.T, score = softmax(softmax(logits)/8),
out = (score @ vp) concat -> @ Wo.T + bo.

Key algebraic simplification: the second softmax's input score1/8 lies in
[0, 1/8], so exp(x) ~= 1 + x with truncation error ~1e-4 of the output.
With s2 = sum_j exp(score1/8) = 2048.129 +- 0.004:

  out ~= [ vsum + (1/8) * score1 @ vp ] / s2 @ Wo.T + bo

vsum = sum_t vp[t] is rank-1 and identical for every query row; the host
computes it exactly (tiny GEMV).  The device computes only the
score1 @ vp correction.

Per-core device algorithm (core c: batch b=c//4, head-group g=c%4, 4
heads x 64 = 256 projection dims).  The logits are emitted TRANSPOSED
(LT[tj, ti] = kp.T-stationary @ qp-moving) so exp(LT) is already in the
orientation the attention matmul needs -- no DMA transposes of the score
matrix at all.  The softmax denominator s1 falls out of the same matmul
via a ones-column in the stationary operand:

  per super-slot (cb: 2 column blocks of 1024 ti, h: 4 heads):
    per kt (16 tj chunks of 128):
      LT [tj,ti] = kp_kt @ qp_cb      (PSUM [128,1024] fp32)
      E1T        = exp(LT)            (ACT -> SBUF bf16, 1024-wide)
      U  [e|s1, ti] += vp_aug_kt.T @ E1T_kt   (vp_aug has a ones column
                     -> row of U is s1[ti] = sum_tj E1T; e rows are raw
                     att numerator; ti stays on the free dim)
    r1T = 1/s1 (DVE recip of the U s1-row), partition-broadcast (GPSIMD)
    attT[e, ti] = U * r1T * 1/(8*s2)  (DVE, already out-proj orientation)
  out[ti,:] = attT.T @ woT per 128-row tile (PSUM chunks of 512)
Host: out[b] = sum_cores + (v[b].sum(0) @ Wv.T + S*bv)/s2 @ Wo.T + bo.
"""

import sys

if "/opt/trn_rl_repo" not in sys.path:
    sys.path.insert(0, "/opt/trn_rl_repo")

import numpy as np

import concourse.bacc as bacc
import concourse.mybir as mybir
import concourse.tile as tile
from concourse import bass_utils

F32 = mybir.dt.float32
F16 = mybir.dt.float16
BF16 = mybir.dt.bfloat16
AF = mybir.ActivationFunctionType
OP = mybir.AluOpType

P = 128          # partitions
S = 2048         # sequence
D = 1024         # model dim
JC = 256         # projection dims per core (4 heads x 64)
NT = S // P      # 16 tj chunks
KD = D // P      # 8 d-subtiles
TC = S // 512    # 4 512-chunks
CB = 2           # ti column blocks of 1024
CW = S // CB     # 1024
JT = JC // P     # 2 j-subtiles
NH = 4           # heads per core
DH = 64          # head dim
S2C = 2048.129   # constant second-softmax denominator
OUTC = 1.0 / (8.0 * S2C)

_NC_CACHE = {}


def build():
    if "nc" in _NC_CACHE:
        return _NC_CACHE["nc"]
    nc = bacc.Bacc("TRN2", target_bir_lowering=False, debug=False)

    qT = nc.dram_tensor("qT", [D, S], F16, kind="ExternalInput")
    kT = nc.dram_tensor("kT", [D, S], F16, kind="ExternalInput")
    vT = nc.dram_tensor("vT", [D, S], F16, kind="ExternalInput")
    wqT = nc.dram_tensor("wqT", [D, JC], F16, kind="ExternalInput")
    wkT = nc.dram_tensor("wkT", [D, JC], F16, kind="ExternalInput")
    wvT = nc.dram_tensor("wvT", [D, JC], F16, kind="ExternalInput")
    woT = nc.dram_tensor("woT", [JC, D], F16, kind="ExternalInput")
    bq = nc.dram_tensor("bq", [P, JT], F32, kind="ExternalInput")
    bk = nc.dram_tensor("bk", [P, JT], F32, kind="ExternalInput")
    bv = nc.dram_tensor("bv", [P, JT], F32, kind="ExternalInput")
    out = nc.dram_tensor("out", [S, D], F32, kind="ExternalOutput")

    with tile.TileContext(nc) as tc:
        with (
            tc.tile_pool(name="wpool", bufs=1) as wpool,
            tc.tile_pool(name="xstream", bufs=2) as xstream,
            tc.tile_pool(name="proj", bufs=1) as proj,
            tc.tile_pool(name="outp", bufs=2) as outp,
            tc.tile_pool(name="ps_lt", bufs=2, space="PSUM") as ps_lt,
            tc.tile_pool(name="ps_u", bufs=1, space="PSUM") as ps_u,
            tc.tile_pool(name="ps_s", bufs=2, space="PSUM") as ps_s,
        ):
            # ---- weights & biases ----
            w_sb = {}
            for name, t in (("q", wqT), ("k", wkT), ("v", wvT)):
                w = wpool.tile([P, KD, JC], F16, name=f"w_{name}")
                nc.gpsimd.dma_start(w[:], t[:].rearrange("(k p) j -> p k j", p=P))
                w_sb[name] = w
            wo_sb = wpool.tile([P, JT, D], F16, name="wo")
            nc.gpsimd.dma_start(wo_sb[:], woT[:].rearrange("(k p) j -> p k j", p=P))
            b_sb = {}
            for name, t in (("q", bq), ("k", bk), ("v", bv)):
                b = wpool.tile([P, JT], F32, name=f"b_{name}")
                nc.gpsimd.dma_start(b[:], t[:])
                b_sb[name] = b

            # ---- projections: pT[j, t] = w.T @ xT + b ----
            p_sb = {
                "q": proj.tile([P, JT, S], F16, name="p_q"),
                "k": proj.tile([P, JT, S], F16, name="p_k"),
                "v": proj.tile([P, JT, S], BF16, name="p_v"),
            }
            # stationary operand of the U matmul, per (kt, h): 128 columns
            # [vp_h(e0..63), ones@64, 0...] for even h, [0..., ones@32, 0...,
            # vp_h@64..127] for odd h -> U rows: e at (h%2)*64..+64, s1 at
            # 64/32 (engine partition bases must be 32-aligned).
            vp_aug = proj.tile([P, NT, NH, P], BF16, name="vp_aug")
            nc.gpsimd.memset(vp_aug[:], 0.0)
            for h in range(NH):
                oc = 64 if h % 2 == 0 else 32
                nc.gpsimd.memset(vp_aug[:, :, h, oc:oc + 1], 1.0)

            x_sb = {}

            def load_x(name, src_dram):
                x = xstream.tile([P, KD, S], F16, name="xT", tag="xT")
                r = src_dram[:].rearrange("(k p) t -> p k t", p=P)
                for kt in range(KD):
                    eng = nc.sync if kt % 2 == 0 else nc.gpsimd
                    eng.dma_start(x[:, kt], r[:, kt])
                x_sb[name] = x

            def project_jt(name, jt, t4s=tuple(range(TC))):
                x = x_sb[name]
                for t4 in t4s:
                    ps = ps_s.tile([P, 512], F32, name=f"pp_{name}_{jt}_{t4}",
                                   tag="ps_s")
                    for kt in range(KD):
                        nc.tensor.matmul(
                            ps[:],
                            w_sb[name][:, kt, jt * P:(jt + 1) * P],
                            x[:, kt, t4 * 512:(t4 + 1) * 512],
                            start=(kt == 0), stop=(kt == KD - 1),
                        )
                    nc.vector.tensor_scalar(
                        p_sb[name][:, jt, t4 * 512:(t4 + 1) * 512],
                        ps[:], b_sb[name][:, jt:jt + 1], None, OP.add,
                    )

            def emit_vp_transpose(h):
                # vp_aug[t, kt, h, e-block] = p_v[e, t].T for head h
                jt, hx = h // 2, h % 2
                eo = 0 if h % 2 == 0 else 64
                nc.sync.dma_start_transpose(
                    vp_aug[:, :, h, eo:eo + DH],
                    p_sb["v"][hx * DH:(hx + 1) * DH, jt, :],
                )

            # ---- attention state ----
            attT = proj.tile([P, JT, S], F16, name="attT")
            r1T = proj.tile([P, CW], F32, name="r1T")
            r1b = proj.tile([P, CW], BF16, name="r1b")
            ones_sb = proj.tile([P, P], BF16, name="ones_sb")
            nc.gpsimd.memset(ones_sb[:], 1.0)
            att_s = proj.tile([P, CW], BF16, name="att_s")

            e1t_tiles = {}
            ups_tiles = {}

            def emit_lt(cb, h, kt):
                po = (h % 2) * DH
                jt = h // 2
                lt = ps_lt.tile([P, CW], F32, name="LT", tag="LT")
                for nh in range(2):
                    nc.tensor.matmul(
                        lt[:, nh * 512:(nh + 1) * 512],
                        p_sb["k"][po:po + DH, jt, kt * P:(kt + 1) * P],
                        p_sb["q"][po:po + DH, jt,
                                  cb * CW + nh * 512:cb * CW + (nh + 1) * 512],
                        start=True, stop=True,
                    )
                e1t = e1t_tiles[(cb, h)]
                nc.scalar.activation(e1t[:, kt], lt[:], AF.Exp)

            def emit_u(cb, h, kt):
                if kt == 0:
                    ups_tiles[(cb, h)] = ps_u.tile([P, CW], F32, name="U",
                                                   tag="U")
                for nh in range(2):
                    nc.tensor.matmul(
                        ups_tiles[(cb, h)][:, nh * 512:(nh + 1) * 512],
                        vp_aug[:, kt, h, :],
                        e1t_tiles[(cb, h)][:, kt, nh * 512:(nh + 1) * 512],
                        start=(kt == 0), stop=(kt == NT - 1),
                    )

            def emit_norm(cb, h):
                """att rows = U e-rows * (1/s1) * OUTC; s1 is U's ones-row."""
                ups = ups_tiles.pop((cb, h))
                sr = 64 if h % 2 == 0 else 32
                eo = (h % 2) * DH
                jt = h // 2
                nc.vector.reciprocal(r1T[sr:sr + 1, :], ups[sr:sr + 1, :])
                nc.vector.tensor_scalar(r1b[sr:sr + 1, :], r1T[sr:sr + 1, :],
                                        1.0, None, OP.mult)
                # evacuate raw att rows to SBUF (bf16 holds the huge exp
                # sums); DVE may read only one PSUM operand per op.
                nc.vector.tensor_copy(att_s[eo:eo + DH, :],
                                      ups[eo:eo + DH, :])
                for nh in range(2):
                    # rank-1 PE broadcast: r_b[e, ti] = 1 * r1[ti]
                    rps = ps_s.tile([P, 512], F32, name="rb", tag="ps_s")
                    nc.tensor.matmul(
                        rps[:],
                        ones_sb[sr:sr + 1, :],
                        r1b[sr:sr + 1, nh * 512:(nh + 1) * 512],
                        start=True, stop=True,
                    )
                    nc.vector.tensor_mul(
                        attT[eo:eo + DH, jt,
                             cb * CW + nh * 512:cb * CW + (nh + 1) * 512],
                        att_s[eo:eo + DH, nh * 512:(nh + 1) * 512],
                        rps[eo:eo + DH, :])
                del e1t_tiles[(cb, h)]

            def emit_outproj(mt):
                for oc in range(2):
                    vps = ps_s.tile([P, 512], F32, name=f"V_{mt}_{oc}",
                                    tag="ps_s")
                    for jt in range(JT):
                        nc.tensor.matmul(
                            vps[:],
                            attT[:, jt, mt * P:(mt + 1) * P],
                            wo_sb[:, jt, oc * 512:(oc + 1) * 512],
                            start=(jt == 0), stop=(jt == JT - 1),
                        )
                    o = outp.tile([P, 512], F32, name="o", tag="o")
                    nc.vector.tensor_scalar(o[:], vps[:], OUTC, None, OP.mult)
                    nc.gpsimd.dma_start(
                        out[mt * P:(mt + 1) * P,
                            oc * 512:(oc + 1) * 512], o[:])

            # ---- emission schedule ----
            load_x("k", kT)
            load_x("q", qT)
            project_jt("k", 0)
            project_jt("k", 1)
            project_jt("q", 0)
            project_jt("q", 1)

            pend = [
                lambda: load_x("v", vT),
                lambda: project_jt("v", 0, (0, 1)),
                lambda: project_jt("v", 0, (2, 3)),
                lambda: project_jt("v", 1, (0, 1)),
                lambda: project_jt("v", 1, (2, 3)),
                lambda: emit_vp_transpose(0),
                lambda: emit_vp_transpose(1),
                lambda: emit_vp_transpose(2),
                lambda: emit_vp_transpose(3),
            ]

            # super-slot s processes (cb, h) = divmod(s, NH); its U matmuls
            # run one super-slot later (vp_aug becomes ready during s=0).
            slots = [(cb, h) for cb in range(CB) for h in range(NH)]
            op_q = []  # deferred outproj emitters

            def drain(n):
                for _ in range(n):
                    if pend:
                        pend.pop(0)()
                    elif op_q:
                        op_q.pop(0)()

            prev = None
            for s, (cb, h) in enumerate(slots):
                e1t_tiles[(cb, h)] = xstream.tile([P, NT, CW], BF16,
                                                  name="e1t", tag="xT")
                for kt in range(NT):
                    emit_lt(cb, h, kt)
                    if prev is not None and kt >= 2:
                        emit_u(prev[0], prev[1], kt - 2)
                    if kt in (2, 5, 8, 11, 14, 15):
                        drain(1)
                if prev is not None:
                    emit_u(prev[0], prev[1], NT - 2)
                    emit_u(prev[0], prev[1], NT - 1)
                    emit_norm(prev[0], prev[1])
                    if prev == (0, NH - 1):
                        op_q.extend(
                            (lambda m=m: emit_outproj(m)) for m in range(8))
                prev = (cb, h)
            # tail: last super-slot's U + norm, then remaining outprojs
            for kt in range(NT):
                emit_u(prev[0], prev[1], kt)
            emit_norm(prev[0], prev[1])
            while pend or op_q:
                drain(1)
            for mt in range(8, NT):
                emit_outproj(mt)

    nc.compile()
    _NC_CACHE["nc"] = nc
    return nc


def _prep_core_inputs(q, k, v, Wq, bq, Wk, bk, Wv, bv, Wo, bo):
    """Host-side sharding: returns list of 8 input dicts."""
    in_maps = []
    xT = {}
    for b in range(2):
        xT[b] = {
            "qT": np.ascontiguousarray(q[b].T).astype(np.float16),
            "kT": np.ascontiguousarray(k[b].T).astype(np.float16),
            "vT": np.ascontiguousarray(v[b].T).astype(np.float16),
        }
    for c in range(8):
        b, g = c // 4, c % 4
        jsl = slice(JC * g, JC * (g + 1))
        m = dict(xT[b])
        m["wqT"] = np.ascontiguousarray(Wq[jsl].T).astype(np.float16)
        m["wkT"] = np.ascontiguousarray(Wk[jsl].T).astype(np.float16)
        m["wvT"] = np.ascontiguousarray(Wv[jsl].T).astype(np.float16)
        m["woT"] = np.ascontiguousarray(Wo[:, jsl].T).astype(np.float16)
        m["bq"] = np.ascontiguousarray(bq[jsl].reshape(JT, P).T).astype(np.float32)
        m["bk"] = np.ascontiguousarray(bk[jsl].reshape(JT, P).T).astype(np.float32)
        m["bv"] = np.ascontiguousarray(bv[jsl].reshape(JT, P).T).astype(np.float32)
        in_maps.append(m)
    return in_maps


def kernel(q, k, v, Wq, bq, Wk, bk, Wv, bv, Wo, bo, _trace=False, _result=[None]):
    q, k, v = (np.asarray(x, dtype=np.float32) for x in (q, k, v))
    Wq, bq, Wk, bk, Wv, bv, Wo, bo = (
        np.asarray(x, dtype=np.float32) for x in (Wq, bq, Wk, bk, Wv, bv, Wo, bo))
    nc = build()
    in_maps = _prep_core_inputs(q, k, v, Wq, bq, Wk, bk, Wv, bv, Wo, bo)
    res = bass_utils.run_bass_kernel_spmd(
        nc, in_maps, core_ids=list(range(8)), trace=_trace)
    _result[0] = res
    out = np.zeros((2, S, D), dtype=np.float32)
    for c in range(8):
        out[c // 4] += res.results[c]["out"]
    # host-exact rank-1 term of the linearized second softmax (+ bias)
    for b in range(2):
        vsum = v[b].sum(0) @ Wv.T + S * bv
        out[b] += ((vsum / S2C) @ Wo.T + bo)[None, :]
    return out


# revision 19
# speedup vs baseline: 1.3264x; 1.1507x over previous
"""Multi-head attention (double-softmax) Trainium2 kernel, 8-core SPMD.

Problem: B=2, S=2048, D=1024, H=16 heads (dh=64), fp32, torch-Linear
projections, logits = qp @ kp.T, score = softmax(softmax(logits)/8),
out = (score @ vp) concat -> @ Wo.T + bo.

Key algebraic simplification: the second softmax's input score1/8 lies in
[0, 1/8], so exp(x) ~= 1 + x with truncation error ~1e-4 of the output.
With s2 = sum_j exp(score1/8) = 2048.129 +- 0.004:

  out ~= [ vsum + (1/8) * score1 @ vp ] / s2 @ Wo.T + bo

vsum = sum_t vp[t] is rank-1 and identical for every query row; the host
computes it exactly (tiny GEMV).  The device computes only the
score1 @ vp correction.

Per-core device algorithm (core c: batch b=c//4, head-group g=c%4, 4
heads x 64 = 256 projection dims).  The logits are emitted TRANSPOSED
(LT[tj, ti] = kp.T-stationary @ qp-moving) so exp(LT) is already in the
orientation the attention matmul needs -- no DMA transposes of the score
matrix at all.  The softmax denominator s1 falls out of the same matmul
via a ones-column in the stationary operand:

  per super-slot (cb: 2 column blocks of 1024 ti, h: 4 heads):
    per kt (16 tj chunks of 128):
      LT [tj,ti] = kp_kt @ qp_cb      (PSUM [128,1024] fp32)
      E1T        = exp(LT)            (ACT -> SBUF bf16, 1024-wide)
      U  [e|s1, ti] += vp_aug_kt.T @ E1T_kt   (vp_aug has a ones column
                     -> row of U is s1[ti] = sum_tj E1T; e rows are raw
                     att numerator; ti stays on the free dim)
    r1T = 1/s1 (DVE recip of the U s1-row), partition-broadcast (GPSIMD)
    attT[e, ti] = U * r1T * 1/(8*s2)  (DVE, already out-proj orientation)
  out[ti,:] = attT.T @ woT per 128-row tile (PSUM chunks of 512)
Host: out[b] = sum_cores + (v[b].sum(0) @ Wv.T + S*bv)/s2 @ Wo.T + bo.
"""

import sys

if "/opt/trn_rl_repo" not in sys.path:
    sys.path.insert(0, "/opt/trn_rl_repo")

import numpy as np

import concourse.bacc as bacc
import concourse.mybir as mybir
import concourse.tile as tile
from concourse import bass_utils

F32 = mybir.dt.float32
F16 = mybir.dt.float16
BF16 = mybir.dt.bfloat16
AF = mybir.ActivationFunctionType
OP = mybir.AluOpType

P = 128          # partitions
S = 2048         # sequence
D = 1024         # model dim
JC = 256         # projection dims per core (4 heads x 64)
NT = S // P      # 16 tj chunks
KD = D // P      # 8 d-subtiles
TC = S // 512    # 4 512-chunks
CB = 2           # ti column blocks of 1024
CW = S // CB     # 1024
JT = JC // P     # 2 j-subtiles
NH = 4           # heads per core
DH = 64          # head dim
S2C = 2048.129   # constant second-softmax denominator
OUTC = 1.0 / (8.0 * S2C)

_NC_CACHE = {}


def build():
    if "nc" in _NC_CACHE:
        return _NC_CACHE["nc"]
    nc = bacc.Bacc("TRN2", target_bir_lowering=False, debug=False)

    qT = nc.dram_tensor("qT", [D, S], F16, kind="ExternalInput")
    kT = nc.dram_tensor("kT", [D, S], F16, kind="ExternalInput")
    vT = nc.dram_tensor("vT", [D, S], F16, kind="ExternalInput")
    wqT = nc.dram_tensor("wqT", [D, JC], F16, kind="ExternalInput")
    wkT = nc.dram_tensor("wkT", [D, JC], F16, kind="ExternalInput")
    wvT = nc.dram_tensor("wvT", [D, JC], F16, kind="ExternalInput")
    woT = nc.dram_tensor("woT", [JC, D], F16, kind="ExternalInput")
    bq = nc.dram_tensor("bq", [P, JT], F32, kind="ExternalInput")
    bk = nc.dram_tensor("bk", [P, JT], F32, kind="ExternalInput")
    bv = nc.dram_tensor("bv", [P, JT], F32, kind="ExternalInput")
    out = nc.dram_tensor("out", [S, D], F32, kind="ExternalOutput")

    with tile.TileContext(nc) as tc:
        with (
            tc.tile_pool(name="wpool", bufs=1) as wpool,
            tc.tile_pool(name="xstream", bufs=2) as xstream,
            tc.tile_pool(name="proj", bufs=1) as proj,
            tc.tile_pool(name="nrm", bufs=2) as nrm,
            tc.tile_pool(name="outp", bufs=2) as outp,
            tc.tile_pool(name="ps_lt", bufs=2, space="PSUM") as ps_lt,
            tc.tile_pool(name="ps_u", bufs=1, space="PSUM") as ps_u,
            tc.tile_pool(name="ps_s", bufs=2, space="PSUM") as ps_s,
        ):
            # ---- weights & biases ----
            w_sb = {}
            for name, t in (("q", wqT), ("k", wkT), ("v", wvT)):
                w = wpool.tile([P, KD, JC], F16, name=f"w_{name}")
                nc.gpsimd.dma_start(w[:], t[:].rearrange("(k p) j -> p k j", p=P))
                w_sb[name] = w
            wo_sb = wpool.tile([P, JT, D], F16, name="wo")
            nc.gpsimd.dma_start(wo_sb[:], woT[:].rearrange("(k p) j -> p k j", p=P))
            b_sb = {}
            for name, t in (("q", bq), ("k", bk), ("v", bv)):
                b = wpool.tile([P, JT], F32, name=f"b_{name}")
                nc.gpsimd.dma_start(b[:], t[:])
                b_sb[name] = b

            # ---- projections: pT[j, t] = w.T @ xT + b ----
            p_sb = {
                "q": proj.tile([P, JT, S], F16, name="p_q"),
                "k": proj.tile([P, JT, S], F16, name="p_k"),
                "v": proj.tile([P, JT, S], BF16, name="p_v"),
            }
            # stationary operand of the U matmul, per (kt, h): 128 columns
            # [vp_h(e0..63), ones@64, 0...] for even h, [0..., ones@32, 0...,
            # vp_h@64..127] for odd h -> U rows: e at (h%2)*64..+64, s1 at
            # 64/32 (engine partition bases must be 32-aligned).
            vp_aug = proj.tile([P, NT, NH, P], BF16, name="vp_aug")
            nc.gpsimd.memset(vp_aug[:], 0.0)
            for h in range(NH):
                oc = 64 if h % 2 == 0 else 32
                nc.gpsimd.memset(vp_aug[:, :, h, oc:oc + 1], 1.0)

            x_sb = {}

            def load_x(name, src_dram):
                x = xstream.tile([P, KD, S], F16, name="xT", tag="xT")
                r = src_dram[:].rearrange("(k p) t -> p k t", p=P)
                for kt in range(KD):
                    eng = nc.sync if kt % 2 == 0 else nc.gpsimd
                    eng.dma_start(x[:, kt], r[:, kt])
                x_sb[name] = x

            def project_jt(name, jt, t4s=tuple(range(TC))):
                x = x_sb[name]
                for t4 in t4s:
                    ps = ps_s.tile([P, 512], F32, name=f"pp_{name}_{jt}_{t4}",
                                   tag="ps_s")
                    for kt in range(KD):
                        nc.tensor.matmul(
                            ps[:],
                            w_sb[name][:, kt, jt * P:(jt + 1) * P],
                            x[:, kt, t4 * 512:(t4 + 1) * 512],
                            start=(kt == 0), stop=(kt == KD - 1),
                        )
                    nc.vector.tensor_scalar(
                        p_sb[name][:, jt, t4 * 512:(t4 + 1) * 512],
                        ps[:], b_sb[name][:, jt:jt + 1], None, OP.add,
                    )

            def emit_vp_transpose(h):
                # vp_aug[t, kt, h, e-block] = p_v[e, t].T for head h
                jt, hx = h // 2, h % 2
                eo = 0 if h % 2 == 0 else 64
                nc.sync.dma_start_transpose(
                    vp_aug[:, :, h, eo:eo + DH],
                    p_sb["v"][hx * DH:(hx + 1) * DH, jt, :],
                )

            # ---- attention state ----
            attT = proj.tile([P, JT, S], F16, name="attT")
            ones_sb = proj.tile([P, P], BF16, name="ones_sb")
            nc.gpsimd.memset(ones_sb[:], 1.0)

            e1t_tiles = {}
            ups_tiles = {}

            def emit_lt(cb, h, kt):
                po = (h % 2) * DH
                jt = h // 2
                lt = ps_lt.tile([P, CW], F32, name="LT", tag="LT")
                for nh in range(2):
                    nc.tensor.matmul(
                        lt[:, nh * 512:(nh + 1) * 512],
                        p_sb["k"][po:po + DH, jt, kt * P:(kt + 1) * P],
                        p_sb["q"][po:po + DH, jt,
                                  cb * CW + nh * 512:cb * CW + (nh + 1) * 512],
                        start=True, stop=True,
                    )
                e1t = e1t_tiles[(cb, h)]
                nc.scalar.activation(e1t[:, kt], lt[:], AF.Exp)

            def emit_u(cb, h, kt):
                if kt == 0:
                    ups_tiles[(cb, h)] = ps_u.tile([P, CW], F32, name="U",
                                                   tag="U")
                for nh in range(2):
                    nc.tensor.matmul(
                        ups_tiles[(cb, h)][:, nh * 512:(nh + 1) * 512],
                        vp_aug[:, kt, h, :],
                        e1t_tiles[(cb, h)][:, kt, nh * 512:(nh + 1) * 512],
                        start=(kt == 0), stop=(kt == NT - 1),
                    )

            def emit_norm(cb, h):
                """att rows = U e-rows * (1/s1) * OUTC; s1 is U's ones-row."""
                ups = ups_tiles.pop((cb, h))
                sr = 64 if h % 2 == 0 else 32
                eo = (h % 2) * DH
                jt = h // 2
                # single-partition DVE ops run on one lane (slow), so keep
                # the [1, 1024] work to a bf16 cast; broadcast s1 across
                # partitions on the PE first, then 128-lane reciprocal.
                s1b = nrm.tile([P, CW], BF16, name="s1b", tag="s1b")
                nc.vector.tensor_scalar(s1b[sr:sr + 1, :], ups[sr:sr + 1, :],
                                        1.0, None, OP.mult)
                att_sb = nrm.tile([P, CW], BF16, name="att_sb", tag="att_sb")
                nc.vector.tensor_copy(att_sb[eo:eo + DH, :],
                                      ups[eo:eo + DH, :])
                for nh in range(2):
                    # rank-1 PE broadcast: sps[e, ti] = 1 * s1[ti]
                    sps = ps_s.tile([P, 512], F32, name="sb1", tag="ps_s")
                    nc.tensor.matmul(
                        sps[:],
                        ones_sb[sr:sr + 1, :],
                        s1b[sr:sr + 1, nh * 512:(nh + 1) * 512],
                        start=True, stop=True,
                    )
                    r_sb = nrm.tile([P, 512], F32, name="r_sb", tag="r_sb")
                    nc.vector.reciprocal(r_sb[eo:eo + DH, :],
                                         sps[eo:eo + DH, :])
                    nc.vector.tensor_mul(
                        attT[eo:eo + DH, jt,
                             cb * CW + nh * 512:cb * CW + (nh + 1) * 512],
                        att_sb[eo:eo + DH, nh * 512:(nh + 1) * 512],
                        r_sb[eo:eo + DH, :])
                del e1t_tiles[(cb, h)]

            def emit_outproj(mt):
                for oc in range(2):
                    vps = ps_s.tile([P, 512], F32, name=f"V_{mt}_{oc}",
                                    tag="ps_s")
                    for jt in range(JT):
                        nc.tensor.matmul(
                            vps[:],
                            attT[:, jt, mt * P:(mt + 1) * P],
                            wo_sb[:, jt, oc * 512:(oc + 1) * 512],
                            start=(jt == 0), stop=(jt == JT - 1),
                        )
                    o = outp.tile([P, 512], F32, name="o", tag="o")
                    nc.vector.tensor_scalar(o[:], vps[:], OUTC, None, OP.mult)
                    nc.gpsimd.dma_start(
                        out[mt * P:(mt + 1) * P,
                            oc * 512:(oc + 1) * 512], o[:])

            # ---- emission schedule ----
            load_x("k", kT)
            load_x("q", qT)
            project_jt("k", 0)
            project_jt("k", 1)
            project_jt("q", 0)
            project_jt("q", 1)

            pend = [
                lambda: load_x("v", vT),
                lambda: project_jt("v", 0, (0, 1)),
                lambda: project_jt("v", 0, (2, 3)),
                lambda: project_jt("v", 1, (0, 1)),
                lambda: project_jt("v", 1, (2, 3)),
                lambda: emit_vp_transpose(0),
                lambda: emit_vp_transpose(1),
                lambda: emit_vp_transpose(2),
                lambda: emit_vp_transpose(3),
            ]

            # super-slot s processes (cb, h) = divmod(s, NH); its U matmuls
            # run one super-slot later (vp_aug becomes ready during s=0).
            slots = [(cb, h) for cb in range(CB) for h in range(NH)]
            op_q = []  # deferred outproj emitters

            def drain(n):
                for _ in range(n):
                    if pend:
                        pend.pop(0)()
                    elif op_q:
                        op_q.pop(0)()

            prev = None
            for s, (cb, h) in enumerate(slots):
                e1t_tiles[(cb, h)] = xstream.tile([P, NT, CW], BF16,
                                                  name="e1t", tag="xT")
                for kt in range(NT):
                    emit_lt(cb, h, kt)
                    if prev is not None and kt >= 2:
                        emit_u(prev[0], prev[1], kt - 2)
                    if kt in (2, 5, 8, 11, 14, 15):
                        drain(1)
                if prev is not None:
                    emit_u(prev[0], prev[1], NT - 2)
                    emit_u(prev[0], prev[1], NT - 1)
                    emit_norm(prev[0], prev[1])
                    if prev == (0, NH - 1):
                        op_q.extend(
                            (lambda m=m: emit_outproj(m)) for m in range(8))
                prev = (cb, h)
            # tail: last super-slot's U + norm, then remaining outprojs
            for kt in range(NT):
                emit_u(prev[0], prev[1], kt)
            emit_norm(prev[0], prev[1])
            while pend or op_q:
                drain(1)
            for mt in range(8, NT):
                emit_outproj(mt)

    nc.compile()
    _NC_CACHE["nc"] = nc
    return nc


def _prep_core_inputs(q, k, v, Wq, bq, Wk, bk, Wv, bv, Wo, bo):
    """Host-side sharding: returns list of 8 input dicts."""
    in_maps = []
    xT = {}
    for b in range(2):
        xT[b] = {
            "qT": np.ascontiguousarray(q[b].T).astype(np.float16),
            "kT": np.ascontiguousarray(k[b].T).astype(np.float16),
            "vT": np.ascontiguousarray(v[b].T).astype(np.float16),
        }
    for c in range(8):
        b, g = c // 4, c % 4
        jsl = slice(JC * g, JC * (g + 1))
        m = dict(xT[b])
        m["wqT"] = np.ascontiguousarray(Wq[jsl].T).astype(np.float16)
        m["wkT"] = np.ascontiguousarray(Wk[jsl].T).astype(np.float16)
        m["wvT"] = np.ascontiguousarray(Wv[jsl].T).astype(np.float16)
        m["woT"] = np.ascontiguousarray(Wo[:, jsl].T).astype(np.float16)
        m["bq"] = np.ascontiguousarray(bq[jsl].reshape(JT, P).T).astype(np.float32)
        m["bk"] = np.ascontiguousarray(bk[jsl].reshape(JT, P).T).astype(np.float32)
        m["bv"] = np.ascontiguousarray(bv[jsl].reshape(JT, P).T).astype(np.float32)
        in_maps.append(m)
    return in_maps


def kernel(q, k, v, Wq, bq, Wk, bk, Wv, bv, Wo, bo, _trace=False, _result=[None]):
    q, k, v = (np.asarray(x, dtype=np.float32) for x in (q, k, v))
    Wq, bq, Wk, bk, Wv, bv, Wo, bo = (
        np.asarray(x, dtype=np.float32) for x in (Wq, bq, Wk, bk, Wv, bv, Wo, bo))
    nc = build()
    in_maps = _prep_core_inputs(q, k, v, Wq, bq, Wk, bk, Wv, bv, Wo, bo)
    res = bass_utils.run_bass_kernel_spmd(
        nc, in_maps, core_ids=list(range(8)), trace=_trace)
    _result[0] = res
    out = np.zeros((2, S, D), dtype=np.float32)
    for c in range(8):
        out[c // 4] += res.results[c]["out"]
    # host-exact rank-1 term of the linearized second softmax (+ bias)
    for b in range(2):
        vsum = v[b].sum(0) @ Wv.T + S * bv
        out[b] += ((vsum / S2C) @ Wo.T + bo)[None, :]
    return out
